# revision 1
# baseline (speedup 1.0000x reference)
"""ApsPool3d TRN2 kernel — development version.

Per core (1 batch): input (64, 48, 48, 48) f32 -> output (64, 24, 24, 24) f32.
See design notes in transcript. STAGE env var gates debug outputs.
"""

import os
import sys

for _p in ("/opt/trn_rl_repo", "/root/.axon_site/_ro/trn_rl_repo"):
    if _p not in sys.path:
        sys.path.insert(0, _p)

import numpy as np

import concourse.bass as bass
import concourse.mybir as mybir
import concourse.tile as tile


# ---- inlined tile_patch ----
def _patched_drain_and_barrier(self, tick_clock, wait_clock):
    nc = self.nc
    carrier = mybir.InstNoOp(
        name="tile_drain_wait_carrier",
        engine=mybir.EngineType.SP,
        ins=[],
        outs=[],
    )
    wait_clock.add_sem_waits(
        carrier, tile.ScopedClock({None: tick_clock.global_clock})
    )
    waits = list(carrier.sync_info.on_wait) if carrier.sync_info else []
    for w in waits:
        nop = nc.sync.nop()
        nsi = nop.ins.sync_info
        if nsi is None:
            nop.ins.sync_info = mybir.SyncInfo(on_wait=[w], on_update=[])
        else:
            nsi.on_wait.append(w)
    nc.sync.drain()
    nc.all_engine_barrier()
    assert self.sems is not None
    popped = nc._tile_sem_poison_stack.pop()
    assert popped is self._sem_poison
    nc.clear_and_free_semaphores(list(self.sems.allocated().values()))
    nc.all_engine_barrier()


tile.TileContext._drain_and_barrier = _patched_drain_and_barrier

_SPLIT_SEQ = [0]


def _split_waits(nc, max_waits=1):
    for f in nc.m.functions:
        for bb in f.blocks:
            new_insts = []
            for inst in bb.instructions:
                si = inst.sync_info
                if si is not None and si.on_wait and len(si.on_wait) > max_waits:
                    waits = list(si.on_wait)
                    keep = waits[:max_waits]
                    extras = waits[max_waits:]
                    del si.on_wait[:]
                    si.on_wait.extend(keep)
                    for w in extras:
                        _SPLIT_SEQ[0] += 1
                        nop = mybir.InstNoOp(
                            name=f"waitsplit-{_SPLIT_SEQ[0]}",
                            engine=inst.engine,
                            ins=[],
                            outs=[],
                            sync_info=mybir.SyncInfo(on_wait=[w], on_update=[]),
                        )
                        new_insts.append(nop)
                new_insts.append(inst)
            if len(new_insts) != len(bb.instructions):
                del bb.instructions[:]
                bb.instructions.extend(new_insts)
# ---- end inlined tile_patch ----

from concourse.bass_utils import run_bass_kernel_spmd

F32 = mybir.dt.float32
BF16 = mybir.dt.bfloat16
FP8 = mybir.dt.float8e4
U32 = mybir.dt.uint32
ALU = mybir.AluOpType

C, N = 64, 48
NH = N // 2  # 24
YX = N * N  # 2304
NT = C // 2  # 32 channel-pair tiles
ROWP = 50  # row pitch for u (x data at [2, 50), cols 0-1 zero)
U0P = 50  # u0 row count (rows 0 and 49 zero)
CHUNK_ROWS = 8
NCHUNK = N // CHUNK_ROWS  # 6
GROUP = 2  # tiles per norm-square group
NGROUP = NT // GROUP  # 8
EXT_GROUP = 4  # tiles per extraction/staging group

STAGE = 5


def zperm():
    """m_local (output row) -> z_out. Even z at [0,24), odd at [24,48)."""
    return [2 * i for i in range(NH)] + [2 * i + 1 for i in range(NH)]


def build_weights(filt):
    """W_side/W_center (128,128) f32 and z-parity matrix P (128,2) f32."""
    f = np.asarray(filt[0, 0], dtype=np.float64)
    s = f.sum()
    sz = f.sum(axis=(1, 2)) / s
    sy = f.sum(axis=(0, 2)) / s
    sx = f.sum(axis=(0, 1)) / s
    assert abs(sx[0] - sx[2]) < 1e-12 and abs(sy[0] - sy[2]) < 1e-12
    zp = zperm()
    blk_side = np.zeros((48, 48), dtype=np.float64)
    blk_cent = np.zeros((48, 48), dtype=np.float64)
    for m in range(N):
        z_out = zp[m]
        for dz in (-1, 0, 1):
            z_in = z_out + dz
            if 0 <= z_in < N:
                w = sz[dz + 1] * sy[0]
                blk_side[z_in, m] = w * sx[0]
                blk_cent[z_in, m] = w * sx[1]
    W_side = np.zeros((96, 96), dtype=np.float64)
    W_cent = np.zeros((96, 96), dtype=np.float64)
    for c in range(2):
        W_side[c * N : c * N + N, c * N : c * N + N] = blk_side[:N, :N]
        W_cent[c * N : c * N + N, c * N : c * N + N] = blk_cent[:N, :N]
    P = np.zeros((96, 2), dtype=np.float32)
    for c in range(2):
        P[c * N : c * N + NH, 0] = 1.0
        P[c * N + NH : c * N + N, 1] = 1.0
    return W_side.astype(np.float32), W_cent.astype(np.float32), P


def build_kernel(nc):
    x = nc.declare_dram_parameter("x", [C, N, N, N], F32, isOutput=False)
    w_side_d = nc.declare_dram_parameter("w_side", [96, 96], BF16, isOutput=False)
    w_cent_d = nc.declare_dram_parameter("w_cent", [96, 96], BF16, isOutput=False)
    par_d = nc.declare_dram_parameter("par", [96, 2], F32, isOutput=False)
    out = nc.declare_dram_parameter("out", [C, NH, NH, NH], F32, isOutput=True)
    dbg16 = dbg32 = dbgidx = dbgf = None
    if STAGE == 0:
        dbgf = nc.declare_dram_parameter("dbgf", [128, YX], F32, isOutput=True)
    if STAGE == 1 or STAGE == 2:
        dbg16 = nc.declare_dram_parameter("dbg16", [128, YX], BF16, isOutput=True)
    if STAGE == 3:
        dbg32 = nc.declare_dram_parameter("dbg32", [1, 8], F32, isOutput=True)
        dbgidx = nc.declare_dram_parameter("dbgidx", [1, 8], U32, isOutput=True)


    xf = x.rearrange("c z y x -> c z (y x)")  # (64, 48, 2304)
    of = out.rearrange("c z y x -> c z (y x)")  # (64, 24, 576)

    with tile.TileContext(nc) as tc:
        with (
            tc.tile_pool(name="consts", bufs=1) as consts,
            tc.tile_pool(name="inp", bufs=3) as inp_pool,
            tc.tile_pool(name="work", bufs=1) as work_pool,
            tc.tile_pool(name="ps", bufs=1, space="PSUM") as psum_pool,
            tc.tile_pool(name="store", bufs=1) as store_pool,
            tc.tile_pool(name="stg", bufs=1) as stage_pool,
            tc.tile_pool(name="dramp", bufs=1, space="DRAM") as dram_pool,
        ):
            w_side = consts.tile([96, 96], BF16, tag="ws")
            w_cent = consts.tile([96, 96], BF16, tag="wc")
            par = consts.tile([96, 2], F32, tag="par")
            nc.default_dma_engine.dma_start(w_side[:], w_side_d[:])
            nc.default_dma_engine.dma_start(w_cent[:], w_cent_d[:])
            nc.default_dma_engine.dma_start(par[:], par_d[:])

            stored = store_pool.tile([96, NT * YX + 80], BF16, tag="stored")
            norm_acc = consts.tile([128, NGROUP * 4], F32, tag="nacc")

            # persistent double-buffered work tiles
            t1s = [work_pool.tile([96, (N + 1) * N], BF16, tag=f"t1_{i}", name=f"t1_{i}") for i in range(2)]
            us = [work_pool.tile([128, N * ROWP + 4], BF16, tag=f"u_{i}", name=f"u_{i}") for i in range(2)]
            junks = {
                "v": work_pool.tile([128, GROUP * NH * NH], FP8, tag="junk_v", name="junk_v"),
                "s": work_pool.tile([128, GROUP * NH * NH], FP8, tag="junk_s", name="junk_s"),
                "g": work_pool.tile([128, GROUP * NH * NH], FP8, tag="junk_g", name="junk_g"),
            }
            for i in range(2):
                nc.vector.memset(us[i][:], 0.0)

            psums = [
                psum_pool.tile([128, 3 * 512], F32, tag=f"ps_{i}", name=f"ps_{i}") for i in range(2)
            ]

            for t in range(NT):
                t1 = t1s[t % 2]
                u = us[t % 2]

                # ---- DMA in: 2 channels, partitions c*64 + z ----
                it = inp_pool.tile([128, YX], F32, tag="inp")
                iv = it[0:96].rearrange("(c z) f -> c z f", c=2)
                for cl in range(2):
                    nc.default_dma_engine.dma_start(
                        iv[cl, :, :], xf[2 * t + cl, :, :]
                    )

                if STAGE == 0:
                    if t == 0:
                        nc.default_dma_engine.dma_start(dbgf[:], it[:])
                    continue

                # ---- y box2 #1 from f32 input (fused cast) ----
                # t1[r] = d[r-1] + d[r], r in [0,49); edges are copies
                it96 = it[0:96]
                nc.vector.tensor_copy(t1[:, 0:N], it96[:, 0:N])
                nc.vector.tensor_add(
                    t1[:, N : N * N],
                    it96[:, 0 : (N - 1) * N],
                    it96[:, N : N * N],
                )
                nc.vector.tensor_copy(
                    t1[:, N * N : (N + 1) * N], it96[:, (N - 1) * N : N * N]
                )
                # ---- y box2 #2 ----
                uv = u[0:96, 0 : N * ROWP].rearrange("p (r w) -> p r w", w=ROWP)
                t1v = t1[:, 0 : (N + 1) * N].rearrange("p (r w) -> p r w", w=N)
                nc.vector.tensor_add(
                    uv[:, :, 2 : 2 + N],
                    t1v[:, 0:N, :],
                    t1v[:, 1 : N + 1, :],
                )

                if STAGE == 1:
                    if t == 0:
                        nc.default_dma_engine.dma_start(
                            dbg16[0:96].rearrange("p (r w) -> p r w", w=N),
                            uv[:, :, 2 : 2 + N],
                        )
                    continue

                # ---- PE: 6 chunks x 3 shifted matmuls, 2 psum tiles ----
                for half in range(2):
                    psum = psums[half]
                    for j, (wt, off, st, sp) in enumerate(
                        [
                            (w_side, 1, True, False),
                            (w_side, 3, False, False),
                            (w_cent, 2, False, True),
                        ]
                    ):
                        for ck in range(3):
                            r0 = (half * 3 + ck) * CHUNK_ROWS
                            pout = psum[0:96, ck * 512 : ck * 512 + CHUNK_ROWS * N]
                            a = ROWP * r0 + off
                            rhs = u[0:96, a : a + ROWP * CHUNK_ROWS].rearrange(
                                "p (r w) -> p r w", w=ROWP
                            )[:, :, 0:N]
                            nc.tensor.matmul(pout, wt[:], rhs, start=st, stop=sp)
                    # ---- evac PSUM -> stored bf16 ----
                    sbase = t * YX + half * 3 * CHUNK_ROWS * N
                    sview = stored[0:96, sbase : sbase + 3 * CHUNK_ROWS * N].rearrange(
                        "p (k w) -> p k w", k=3
                    )
                    pview = psum[0:96].rearrange("p (k w) -> p k w", k=3)
                    nc.scalar.copy(sview, pview[:, :, 0 : CHUNK_ROWS * N])

                if STAGE == 2 and t == 0:
                    nc.default_dma_engine.dma_start(dbg16[0:96], stored[0:96, 0:YX])

                # ---- norm squares per completed group ----
                if t % GROUP == GROUP - 1:
                    g = t // GROUP
                    for pc in range(4):
                        xp, yp = pc >> 1, pc & 1
                        v = stored[
                            0:96, g * GROUP * YX : (g + 1) * GROUP * YX
                        ].rearrange("p (tt y x) -> p tt y x", tt=GROUP, y=N)[
                            :, :, yp : N : 2, xp : N : 2
                        ]
                        eng, jk = [
                            (nc.vector, "v"),
                            (nc.vector, "v"),
                            (nc.scalar, "s"),
                            (nc.scalar, "s"),
                        ][pc]
                        jv = junks[jk][0:96].rearrange(
                            "p (tt y x) -> p tt y x", tt=GROUP, y=NH
                        )
                        if eng is nc.scalar:
                            eng.activation(
                                jv,
                                v,
                                mybir.ActivationFunctionType.Square,
                                accum_out=norm_acc[0:96, g * 4 + pc : g * 4 + pc + 1],
                            )
                        else:
                            eng.scalar_tensor_tensor(
                                jv,
                                v,
                                1.0,
                                v,
                                ALU.bypass,
                                ALU.mult,
                                accum_out=norm_acc[0:96, g * 4 + pc : g * 4 + pc + 1],
                            )

            if STAGE <= 2:
                return

            # ---- finalize norms ----
            zred = psum_pool.tile([2, NGROUP * 4], F32, tag="zred")
            nc.tensor.matmul(zred[:], par[:, 0:2], norm_acc[0:96, :], start=True, stop=True)
            zred_s = consts.tile([2, NGROUP * 4], F32, tag="zreds")
            nc.scalar.copy(zred_s[:], zred[:])
            nbounce = dram_pool.tile([2, 4], F32, tag="nbounce", name="nbounce")
            # reduce over groups: (2, pc:4 step 1, g:8 step 4), reduce X
            zv = zred_s[:].rearrange("p (g c) -> p c g", g=NGROUP)
            n8_2 = consts.tile([2, 4], F32, tag="n8_2")
            nc.vector.tensor_reduce(n8_2[:], zv, mybir.AxisListType.X, ALU.add)
            # (2,4) -> (1,8) via DRAM bounce
            nc.default_dma_engine.dma_start(nbounce[:], n8_2[:])
            norms8 = consts.tile([1, 8], F32, tag="norms8")
            nc.default_dma_engine.dma_start(
                norms8[:], nbounce[:].rearrange("z c -> (z c)").rearrange("(o f) -> o f", o=1)
            )
            nmax = consts.tile([1, 8], F32, tag="nmax")
            nidx = consts.tile([1, 8], U32, tag="nidx")
            nc.vector.max(nmax[:], norms8[:])
            nc.vector.max_index(nidx[:], nmax[:], norms8[:])

            if STAGE == 3:
                nc.default_dma_engine.dma_start(dbg32[:], norms8[:])
                nc.default_dma_engine.dma_start(dbgidx[:], nidx[:])
                return

            # ---- registers: p -> offsets ----
            rp = nc.alloc_registers("rp")
            ryx = nc.alloc_registers("ryx")
            rz = nc.alloc_registers("rz")
            rtmp = nc.alloc_registers("rtmp")
            nc.regs_load(rp, nidx[0:1, 0:1])
            nc.regs_alu(rtmp, rp, 1, ALU.bitwise_and)  # dy
            nc.regs_alu(ryx, rtmp, N, ALU.mult)  # 48*dy
            nc.regs_alu(rtmp, rp, 1, ALU.logical_shift_right)
            nc.regs_alu(rtmp, rtmp, 1, ALU.bitwise_and)  # dx
            nc.regs_alu(ryx, ryx, rtmp, ALU.add)  # 48*dy + dx
            nc.regs_alu(rtmp, rp, 2, ALU.logical_shift_right)
            nc.regs_alu(rz, rtmp, 1, ALU.bitwise_and)  # dz
            yx_off = nc.snap(ryx, min_val=0, max_val=49)
            OUTSZ = C * NH * NH * NH
            # sel offset for z-block 0: dz*OUTSZ ; for z-block 1: (1-dz)*OUTSZ
            rs0 = nc.alloc_registers("rs0")
            rs1 = nc.alloc_registers("rs1")
            nc.regs_alu(rs0, rz, OUTSZ, ALU.mult)
            nc.regs_alu(rtmp, rz, 1, ALU.bitwise_xor)
            nc.regs_alu(rs1, rtmp, OUTSZ, ALU.mult)
            sel_off = [
                nc.snap(rs0, min_val=0, max_val=OUTSZ),
                nc.snap(rs1, min_val=0, max_val=OUTSZ),
            ]


            # ---- extraction v3: 8 dynamic cast-copies, dyn-DRAM-dest out ----
            dsel = dram_pool.tile(
                [2 * C * NH * NH * NH], F32, tag="dsel", name="dsel"
            )
            dd_base = [
                dsel[zb * OUTSZ : (zb + 1) * OUTSZ].rearrange(
                    "(c z f) -> c z f", c=C, z=NH
                )
                for zb in range(2)
            ]
            EG = 4
            for g in range(NT // EG):
                src_g = stored[0:96, g * EG * YX : (g + 1) * EG * YX + 73][
                    :, bass.ds(yx_off, EG * YX)
                ].rearrange("p (tt y x) -> p tt y x", tt=EG, y=N)[
                    :, :, 0 : N : 2, 0 : N : 2
                ]
                stg = stage_pool.tile(
                    [96, EG * NH * NH], F32, tag="stg", name=f"stg_{g}"
                )
                dst = stg[:].rearrange("p (tt y x) -> p tt y x", tt=EG, y=NH)
                if g % 2 == 0:
                    nc.scalar.copy(dst, src_g)
                else:
                    nc.vector.tensor_copy(dst, src_g)

                for cl in range(2):
                    for zb in range(2):
                        p0 = cl * N + zb * NH
                        svd = stg[p0 : p0 + NH, :].rearrange(
                            "z (tt f) -> z tt f", tt=EG
                        )
                        c0 = 2 * g * EG + cl
                        dd = dd_base[zb][c0 : c0 + 2 * EG - 1 : 2]
                        nc.default_dma_engine.dma_start(
                            dd.transpose([1, 0, 2]), svd
                        )

            nc.default_dma_engine.dma_start(
                out[:].rearrange("c z y x -> (c z y x)"),
                dsel[bass.ds(sel_off[0], OUTSZ)],
            )


OUT_FINAL_MARK = None


_NC_CACHE = {}


def _get_nc():
    if "nc" not in _NC_CACHE:
        nc = bass.Bass()
        build_kernel(nc)
        _split_waits(nc)
        _NC_CACHE["nc"] = nc
    return _NC_CACHE["nc"]


def run(input_to_pool, filt, trace=False):
    import ml_dtypes

    W_side, W_cent, P = build_weights(np.asarray(filt))
    nc = _get_nc()
    x = np.ascontiguousarray(np.asarray(input_to_pool, dtype=np.float32))
    B = x.shape[0]
    in_maps = []
    for b in range(B):
        in_maps.append(
            {
                "x": x[b],
                "w_side": W_side.astype(ml_dtypes.bfloat16),
                "w_cent": W_cent.astype(ml_dtypes.bfloat16),
                "par": P,
            }
        )
    res = run_bass_kernel_spmd(nc, in_maps, core_ids=list(range(B)), trace=trace)
    outs = np.stack([res.results[b]["out"] for b in range(B)], axis=0)
    return outs, res


def kernel(input_to_pool, filt, permute_indices=None):
    """Full-input entry point: (8,64,48,48,48) f32 -> (8,64,24,24,24) f32."""
    outs, _ = run(input_to_pool, filt, trace=False)
    return outs





# revision 6
# speedup vs baseline: 1.0865x; 1.0865x over previous
"""ApsPool3d TRN2 kernel v10.

Per core (1 batch): input (64, 48, 48, 48) f32 -> output (64, 24, 24, 24) f32.
Pipeline per channel-pair tile (32 tiles, partitions p = c*48+z, free (y,x)):
  DMA in (f32 via SP hwdge; every 4th tile bf16 via gpsimd casting DMA)
  y-blur: 2 flat TT adds (+2 edge-row adds)         [DVE]
  x-blur: 2 flat TT adds + 2 edge-col repairs       [DVE]
  z-blur: 5 matmuls vs block-diag W (bf16)          [PE]
  evac PSUM->stored bf16 (2 half-tiles)             [Act/DVE alternating]
  squares + accum per (2-tile group, yx parity)     [Act]
argmax via P-matmul + reduce + max_index; extraction staged f32 then
predicated DMAs (cond on z-parity) straight to out. STAGE env gates
debug outputs.
"""

import os
import sys

for _p in ("/opt/trn_rl_repo", "/root/.axon_site/_ro/trn_rl_repo"):
    if _p not in sys.path:
        sys.path.insert(0, _p)

import numpy as np

import concourse.bass as bass
import concourse.mybir as mybir
import concourse.tile as tile


# ---- inlined tile_patch ----
def _patched_drain_and_barrier(self, tick_clock, wait_clock):
    nc = self.nc
    carrier = mybir.InstNoOp(
        name="tile_drain_wait_carrier",
        engine=mybir.EngineType.SP,
        ins=[],
        outs=[],
    )
    wait_clock.add_sem_waits(
        carrier, tile.ScopedClock({None: tick_clock.global_clock})
    )
    waits = list(carrier.sync_info.on_wait) if carrier.sync_info else []
    for w in waits:
        nop = nc.sync.nop()
        nsi = nop.ins.sync_info
        if nsi is None:
            nop.ins.sync_info = mybir.SyncInfo(on_wait=[w], on_update=[])
        else:
            nsi.on_wait.append(w)
    nc.sync.drain()
    nc.all_engine_barrier()
    assert self.sems is not None
    popped = nc._tile_sem_poison_stack.pop()
    assert popped is self._sem_poison
    nc.clear_and_free_semaphores(list(self.sems.allocated().values()))
    nc.all_engine_barrier()


tile.TileContext._drain_and_barrier = _patched_drain_and_barrier

_SPLIT_SEQ = [0]


def _split_waits(nc, max_waits=1):
    for f in nc.m.functions:
        for bb in f.blocks:
            new_insts = []
            for inst in bb.instructions:
                si = inst.sync_info
                if si is not None and si.on_wait and len(si.on_wait) > max_waits:
                    waits = list(si.on_wait)
                    keep = waits[:max_waits]
                    extras = waits[max_waits:]
                    del si.on_wait[:]
                    si.on_wait.extend(keep)
                    for w in extras:
                        _SPLIT_SEQ[0] += 1
                        nop = mybir.InstNoOp(
                            name=f"waitsplit-{_SPLIT_SEQ[0]}",
                            engine=inst.engine,
                            ins=[],
                            outs=[],
                            sync_info=mybir.SyncInfo(on_wait=[w], on_update=[]),
                        )
                        new_insts.append(nop)
                new_insts.append(inst)
            if len(new_insts) != len(bb.instructions):
                del bb.instructions[:]
                bb.instructions.extend(new_insts)
# ---- end inlined tile_patch ----

from concourse.bass_utils import run_bass_kernel_spmd

F32 = mybir.dt.float32
BF16 = mybir.dt.bfloat16
FP8 = mybir.dt.float8e4
U32 = mybir.dt.uint32
ALU = mybir.AluOpType

C, N = 64, 48
NH = N // 2  # 24
YX = N * N  # 2304
NT = C // 2  # 32 channel-pair tiles
GROUP = 2  # tiles per norm-square group
NGROUP = NT // GROUP  # 16
EG = 4  # tiles per extraction/staging group
OUTSZ = C * NH * NH * NH

STAGE = int(os.environ.get("STAGE", "5"))
CAST_MOD = int(os.environ.get("CAST_MOD", "4"))  # t % CAST_MOD == CAST_MOD-1 -> gpsimd cast dma
EVAC_DVE_MOD = int(os.environ.get("EVAC_DVE_MOD", "2"))  # t % mod == 1 -> DVE evac


def zperm():
    """j (output partition z-slot) -> z_out. Even z at [0,24), odd at [24,48)."""
    return [2 * i for i in range(NH)] + [2 * i + 1 for i in range(NH)]


def build_weights(filt):
    """W (96,96) bf16 z-blur with permuted z_out and full 1/64 norm; P (96,2) f32."""
    f = np.asarray(filt[0, 0], dtype=np.float64)
    s = f.sum()  # 64 (pre-normalized to sum 1 -> s=1)
    kz = f.sum(axis=(1, 2)) / s  # [.25,.5,.25]
    zp = zperm()
    blk = np.zeros((N, N), dtype=np.float64)
    for m in range(N):
        z_out = zp[m]
        for dz in (-1, 0, 1):
            z_in = z_out + dz
            if 0 <= z_in < N:
                blk[z_in, m] = kz[dz + 1] / 16.0  # (1/4 y) * (1/4 x)
    W = np.zeros((96, 96), dtype=np.float64)
    for c in range(2):
        W[c * N : (c + 1) * N, c * N : (c + 1) * N] = blk
    P = np.zeros((96, 2), dtype=np.float32)
    for c in range(2):
        P[c * N : c * N + NH, 0] = 1.0
        P[c * N + NH : c * N + N, 1] = 1.0
    return W.astype(np.float32), P


def build_kernel(nc):
    x = nc.declare_dram_parameter("x", [C, N, YX], F32, isOutput=False)
    w_d = nc.declare_dram_parameter("w", [96, 96], BF16, isOutput=False)
    par_d = nc.declare_dram_parameter("par", [96, 2], F32, isOutput=False)
    out = nc.declare_dram_parameter("out", [C, NH, NH * NH], F32, isOutput=True)
    dbg16 = dbg32 = dbgidx = None
    if STAGE in (1, 2, 3):
        dbg16 = nc.declare_dram_parameter("dbg16", [96, 2 * YX], BF16, isOutput=True)
    if STAGE == 4:
        dbg32 = nc.declare_dram_parameter("dbg32", [1, 8], F32, isOutput=True)
        dbgidx = nc.declare_dram_parameter("dbgidx", [1, 8], U32, isOutput=True)

    with tile.TileContext(nc) as tc:
        with (
            tc.tile_pool(name="consts", bufs=1) as consts,
            tc.tile_pool(name="inp", bufs=1) as inp_pool,
            tc.tile_pool(name="work", bufs=1) as work_pool,
            tc.tile_pool(name="ps", bufs=1, space="PSUM") as psum_pool,
            tc.tile_pool(name="store", bufs=1) as store_pool,
            tc.tile_pool(name="dramp", bufs=1, space="DRAM") as dram_pool,
        ):
            w = consts.tile([96, 96], BF16, tag="w")
            par = consts.tile([96, 2], F32, tag="par")
            nc.default_dma_engine.dma_start(w[:], w_d[:])
            nc.default_dma_engine.dma_start(par[:], par_d[:])

            stored = store_pool.tile([96, NT * YX + 80], BF16, tag="stored")
            norm_acc = consts.tile([128, NGROUP * 4], F32, tag="nacc")

            # rotating buffers
            ins32 = [inp_pool.tile([96, YX], F32, tag=f"i32_{i}", name=f"i32_{i}") for i in range(2)]
            ins16 = [inp_pool.tile([96, YX], BF16, tag="i16_0", name="i16_0")]
            t1 = work_pool.tile([96, YX + N], BF16, tag="t1", name="t1")
            u = work_pool.tile([96, YX], BF16, tag="u", name="u")
            sx = work_pool.tile([96, YX], BF16, tag="sx", name="sx")
            ws = [work_pool.tile([96, YX], BF16, tag="w_0", name="wb_0")]
            junk = work_pool.tile([96, GROUP * 576], FP8, tag="junk", name="junk")

            psums = [
                psum_pool.tile([128, 1536], F32, tag=f"ps_{i}", name=f"ps_{i}")
                for i in range(2)
            ]

            for t in range(NT):
                cast = (t % CAST_MOD) == CAST_MOD - 1
                # ---- DMA in ----
                src = x[2 * t : 2 * t + 2].rearrange("c z f -> (c z) f")
                if cast:
                    it = ins16[0]
                    nc.gpsimd.dma_start(it[:], src)
                    d = it[:]
                else:
                    it = ins32[(t - t // CAST_MOD) % 2]
                    nc.default_dma_engine.dma_start(it[:], src)
                    d = it[:]

                # ---- y blur (zero-pad): t1[0]=d[0]; t1[r]=d[r-1]+d[r]; t1[48]=d[47] ----
                nc.vector.tensor_copy(t1[:, 0:N], d[:, 0:N])
                nc.vector.tensor_add(t1[:, N:YX], d[:, 0 : YX - N], d[:, N:YX])
                nc.vector.tensor_copy(t1[:, YX : YX + N], d[:, YX - N : YX])
                # u[y] = t1[y] + t1[y+1], all 48 rows flat
                nc.vector.tensor_add(u[:], t1[:, 0:YX], t1[:, N : YX + N])

                # ---- x blur: s[a] = u[a] + u[a+1] (a in [0,2303)) ----
                nc.vector.tensor_add(sx[:, 0 : YX - 1], u[:, 0 : YX - 1], u[:, 1:YX])
                wt = ws[0]
                # w[a] = s[a-1] + s[a], a in [1,2303)
                nc.vector.tensor_add(
                    wt[:, 1 : YX - 1], sx[:, 0 : YX - 2], sx[:, 1 : YX - 1]
                )
                # repairs: w[:, y, 0] = u[:, y, 0] + s[:, y, 0]
                uv = u[:].rearrange("p (y x) -> p y x", x=N)
                sv = sx[:].rearrange("p (y x) -> p y x", x=N)
                wv = wt[:].rearrange("p (y x) -> p y x", x=N)
                nc.vector.tensor_add(wv[:, :, 0:1], uv[:, :, 0:1], sv[:, :, 0:1])
                # w[:, y, 47] = s[:, y, 46] + u[:, y, 47]
                nc.vector.tensor_add(
                    wv[:, :, N - 1 : N], sv[:, :, N - 2 : N - 1], uv[:, :, N - 1 : N]
                )

                if STAGE == 1:
                    if t == 0:
                        nc.default_dma_engine.dma_start(dbg16[0:96, 0:YX], u[:])
                    if t == 1:
                        nc.default_dma_engine.dma_start(dbg16[0:96, YX : 2 * YX], u[:])
                    continue
                if STAGE == 2:
                    if t == 0:
                        nc.default_dma_engine.dma_start(dbg16[0:96, 0:YX], wt[:])
                    if t == 1:
                        nc.default_dma_engine.dma_start(dbg16[0:96, YX : 2 * YX], wt[:])
                    continue

                # ---- PE: z blur, 2 half-tiles x (512+512+128) ----
                for half in range(2):
                    ps = psums[half]
                    base = half * 1152
                    for c0, cw in ((0, 512), (512, 512), (1024, 128)):
                        nc.tensor.matmul(
                            ps[0:96, c0 : c0 + cw],
                            w[:],
                            wt[:, base + c0 : base + c0 + cw],
                            start=True,
                            stop=True,
                        )
                    # ---- evac ----
                    dst = stored[0:96, t * YX + base : t * YX + base + 1152]
                    if EVAC_DVE_MOD > 0 and t % EVAC_DVE_MOD == 1:
                        nc.vector.tensor_copy(dst, ps[0:96, 0:1152])
                    else:
                        nc.scalar.copy(dst, ps[0:96, 0:1152])

                if STAGE == 3:
                    if t == 0:
                        nc.default_dma_engine.dma_start(
                            dbg16[0:96, 0:YX], stored[0:96, 0:YX]
                        )
                    if t == 1:
                        nc.default_dma_engine.dma_start(
                            dbg16[0:96, YX : 2 * YX], stored[0:96, YX : 2 * YX]
                        )

                # ---- norm squares per completed group (Act) ----
                if t % GROUP == GROUP - 1:
                    g = t // GROUP
                    gview = stored[
                        0:96, g * GROUP * YX : (g + 1) * GROUP * YX
                    ].rearrange("p (tt y x) -> p tt y x", tt=GROUP, y=N)
                    jv = junk[:].rearrange("p (tt y x) -> p tt y x", tt=GROUP, y=NH)
                    for pc in range(4):
                        xp, yp = pc >> 1, pc & 1
                        nc.scalar.activation(
                            jv,
                            gview[:, :, yp:N:2, xp:N:2],
                            mybir.ActivationFunctionType.Square,
                            accum_out=norm_acc[0:96, g * 4 + pc : g * 4 + pc + 1],
                        )

            if STAGE <= 2:
                return

            # ---- finalize norms (as baseline) ----
            zred = psum_pool.tile([2, NGROUP * 4], F32, tag="zred")
            nc.tensor.matmul(
                zred[:], par[:, 0:2], norm_acc[0:96, :], start=True, stop=True
            )
            zred_s = consts.tile([2, NGROUP * 4], F32, tag="zreds")
            nc.scalar.copy(zred_s[:], zred[:])
            nbounce = dram_pool.tile([2, 4], F32, tag="nbounce", name="nbounce")
            zv = zred_s[:].rearrange("p (g c) -> p c g", g=NGROUP)
            n8_2 = consts.tile([2, 4], F32, tag="n8_2")
            nc.vector.tensor_reduce(n8_2[:], zv, mybir.AxisListType.X, ALU.add)
            nc.default_dma_engine.dma_start(nbounce[:], n8_2[:])
            norms8 = consts.tile([1, 8], F32, tag="norms8")
            nc.default_dma_engine.dma_start(
                norms8[:],
                nbounce[:].rearrange("z c -> (z c)").rearrange("(o f) -> o f", o=1),
            )
            nmax = consts.tile([1, 8], F32, tag="nmax")
            nidx = consts.tile([1, 8], U32, tag="nidx")
            nc.vector.max(nmax[:], norms8[:])
            nc.vector.max_index(nidx[:], nmax[:], norms8[:])

            if STAGE == 4:
                nc.default_dma_engine.dma_start(dbg32[:], norms8[:])
                nc.default_dma_engine.dma_start(dbgidx[:], nidx[:])
                return

            # ---- registers: phase index -> offsets / conds ----
            rp = nc.alloc_registers("rp")
            ryx = nc.alloc_registers("ryx")
            rz = nc.alloc_registers("rz")
            rtmp = nc.alloc_registers("rtmp")
            nc.regs_load(rp, nidx[0:1, 0:1])
            nc.regs_alu(rtmp, rp, 1, ALU.bitwise_and)  # dy
            nc.regs_alu(ryx, rtmp, N, ALU.mult)  # 48*dy
            nc.regs_alu(rtmp, rp, 1, ALU.logical_shift_right)
            nc.regs_alu(rtmp, rtmp, 1, ALU.bitwise_and)  # dx
            nc.regs_alu(ryx, ryx, rtmp, ALU.add)  # 48*dy + dx
            nc.regs_alu(rtmp, rp, 2, ALU.logical_shift_right)
            nc.regs_alu(rz, rtmp, 1, ALU.bitwise_and)  # dz
            yx_off = nc.snap(ryx, min_val=0, max_val=49)
            # sel offset into dsel for final gather
            rs0 = nc.alloc_registers("rs0")
            nc.regs_alu(rs0, rz, OUTSZ, ALU.mult)
            sel_off = nc.snap(rs0, min_val=0, max_val=OUTSZ)

            # ---- extraction: stage f32, double-write to dsel, dynamic gather ----
            dsel = dram_pool.tile([2 * OUTSZ], F32, tag="dsel", name="dsel")
            dd_base = [
                dsel[zb * OUTSZ : (zb + 1) * OUTSZ].rearrange(
                    "(c z f) -> c z f", c=C, z=NH
                )
                for zb in range(2)
            ]
            stgs = [
                store_pool.tile([96, EG * 576], F32, tag=f"stg{i}", name=f"stg{i}")
                for i in range(2)
            ]
            for g in range(NT // EG):
                src_g = stored[0:96, g * EG * YX : (g + 1) * EG * YX + 73][
                    :, bass.ds(yx_off, EG * YX)
                ].rearrange("p (tt y x) -> p tt y x", tt=EG, y=N)[
                    :, :, 0:N:2, 0:N:2
                ]
                stg = stgs[g % 2]
                dst = stg[:].rearrange("p (tt y x) -> p tt y x", tt=EG, y=NH)
                if g % 2 == 0:
                    nc.scalar.copy(dst, src_g)
                else:
                    nc.vector.tensor_copy(dst, src_g)

                for cl in range(2):
                    for zb in range(2):
                        p0 = cl * N + zb * NH
                        svd = stg[p0 : p0 + NH, :].rearrange(
                            "z (tt f) -> z tt f", tt=EG
                        )
                        c0 = 2 * g * EG + cl
                        dd = dd_base[zb][c0 : c0 + 2 * EG - 1 : 2]
                        eng = nc.default_dma_engine if g % 2 == 0 else nc.scalar
                        eng.dma_start(dd.transpose([1, 0, 2]), svd)

            nc.default_dma_engine.dma_start(
                out[:].rearrange("c z f -> (c z f)"),
                dsel[bass.ds(sel_off, OUTSZ)],
            )


_NC_CACHE = {}


def _get_nc():
    if "nc" not in _NC_CACHE:
        nc = bass.Bass()
        build_kernel(nc)
        _split_waits(nc)
        _NC_CACHE["nc"] = nc
    return _NC_CACHE["nc"]


def run(input_to_pool, filt, trace=False):
    import ml_dtypes

    W, P = build_weights(np.asarray(filt))
    nc = _get_nc()
    x = np.ascontiguousarray(np.asarray(input_to_pool, dtype=np.float32))
    B = x.shape[0]
    in_maps = []
    for b in range(B):
        in_maps.append(
            {
                "x": x[b].reshape(C, N, YX),
                "w": W.astype(ml_dtypes.bfloat16),
                "par": P,
            }
        )
    res = run_bass_kernel_spmd(nc, in_maps, core_ids=list(range(B)), trace=trace)
    outs = np.stack(
        [res.results[b]["out"].reshape(C, NH, NH, NH) for b in range(B)], axis=0
    )
    return outs, res


def kernel(input_to_pool, filt, permute_indices=None):
    """Full-input entry point: (8,64,48,48,48) f32 -> (8,64,24,24,24) f32."""
    outs, _ = run(input_to_pool, filt, trace=False)
    return outs
